# revision 1
# baseline (speedup 1.0000x reference)
"""Multi-head attention TRN2 kernel, v3 (pipelined, low instruction count).

Problem: B=2, T=S=2048, D=1024, H=16, DK=64 (fp32 in/out).

Sharding (8 cores): core i handles batch b = i // 4 and the 4 heads
[4*(i%4), 4*(i%4)+4).  Each core computes q/k/v projections for its head
slice, attention over them, and a *partial* output projection (its heads'
rows of Wo).  The host sums the 4 partials per batch and adds bo.

Design:
  - Single-pass software-pipelined program: unit u = (tcj, h); slot k
    issues scores+exp for unit 4+k, then drains attnV+normalize of units
    LAG behind, with projections/out-projection spread into the slots.
  - x and the q/k/v projection weights are bf16 (host-converted): halves
    input DMA bytes; projection matmuls run bf16 x bf16 at full rate into
    f32 psum; k/q stay f32 after that, so scores are full precision.
  - One DMA instruction per projection unit ([128, 8, 512] bf16, 1MB):
    the shared HWDGE device costs ~625ns per DMA instruction, so few big
    DMAs beat many small ones.
  - tcj0 runs scores g-major (s-chunk-major across heads) so exp is paced
    by the xk DMA stream; v projections ride inside that sweep.
  - Activation engine runs exp ONLY (q/k biases via DVE tensor_scalar_add,
    v bias via DVE tensor_tensor add against a pre-broadcast row).
  - Probs bf16; V carries a ones-column so row/col 64 of the attnV psum
    is the softmax denominator.
  - FLIP=True: attnV in natural [t, dk] layout, V as the 65-wide moving
    operand (65 cyc/matmul); [t,hd]->[hd,t] via DMA-crossbar transpose.
    FLIP=False: V stationary, probs moving (512 cyc/matmul) but 8x fewer
    PE instructions and no transposes.
  - out-proj streams per tcj: psum -> DVE copy -> one DMA per t-row-block.
"""

import numpy as np

B, T, S, D, H, DK = 2, 2048, 2048, 1024, 16, 64
HPC = 4            # heads per core
HD = HPC * DK      # 256 projected cols per core
N_CORES = 8
DC = D // 128      # 8 contraction chunks
TC4 = T // 512     # 4 t-chunks of 512
SC16 = S // 128    # 16 s-chunks of 128
GRP = [(0, 3), (3, 3), (6, 3), (9, 3), (12, 2), (14, 2)]
NG = len(GRP)      # 6 ragged score groups per (h, tcj)
GI = [gi for gi, (st, sz) in enumerate(GRP) for _ in range(sz)]
OFF = [sc - GRP[GI[sc]][0] for sc in range(16)]

FLIP = True
LAG = 2
DVE_EXP = False


def build_core(flip=None):
    import concourse.bass as bass
    import concourse.mybir as mybir
    from concourse import bacc
    from concourse.tile import TileContext

    if flip is None:
        flip = FLIP

    dt = mybir.dt
    f32 = dt.float32
    f32r = dt.float32r
    bf16 = dt.bfloat16
    AF = mybir.ActivationFunctionType

    nc = bacc.Bacc("TRN2", target_bir_lowering=False, debug=False,
                   num_devices=N_CORES)

    wo_dt = bf16 if flip else f32r

    xqT = nc.dram_tensor("xqT", [D, T], bf16, kind="ExternalInput")
    xkT = nc.dram_tensor("xkT", [D, T], bf16, kind="ExternalInput")
    xvT = nc.dram_tensor("xvT", [D, T], bf16, kind="ExternalInput")
    wq = nc.dram_tensor("wq", [D, HD], bf16, kind="ExternalInput")
    wk = nc.dram_tensor("wk", [D, HD], bf16, kind="ExternalInput")
    wv = nc.dram_tensor("wv", [D, HD], bf16, kind="ExternalInput")
    wo = nc.dram_tensor("wo", [HD, D], wo_dt, kind="ExternalInput")
    bqs = nc.dram_tensor("bqs", [HD], f32, kind="ExternalInput")
    bks = nc.dram_tensor("bks", [HD], f32, kind="ExternalInput")
    bvs = nc.dram_tensor("bvs", [HD], f32, kind="ExternalInput")
    out = nc.dram_tensor("out", [T, D], f32, kind="ExternalOutput")

    # whole-unit views: [p 128, c 8, t] so one DMA loads a full unit
    xq_u = xqT.ap().rearrange("(c p) t -> p c t", p=128)
    xk_u = xkT.ap().rearrange("(c p) t -> p c t", p=128)
    xv_u = xvT.ap().rearrange("(c p) t -> p c t", p=128)
    wq_u = wq.ap().rearrange("(c p) n -> p c n", p=128)
    wk_u = wk.ap().rearrange("(c p) n -> p c n", p=128)
    wv_u = wv.ap().rearrange("(c p) n -> p c n", p=128)
    wo_u = wo.ap().rearrange("(c p) n -> p c n", p=128)

    with TileContext(nc) as tc:
        with (
            tc.tile_pool(name="persist", bufs=1) as pp,
            tc.tile_pool(name="xin", bufs=4) as xpool,
            tc.tile_pool(name="xk0", bufs=2) as k0pool,
            tc.tile_pool(name="qt", bufs=2) as qpool,
            tc.tile_pool(name="at", bufs=2) as apool,
            tc.tile_pool(name="probs", bufs=31) as ptpool,
            tc.tile_pool(name="a2", bufs=6) as a2pool,
            tc.tile_pool(name="small", bufs=4) as spool,
            tc.tile_pool(name="rbp", bufs=2) as rbpool,
            tc.tile_pool(name="ostage", bufs=2) as opool,
            tc.tile_pool(name="psS", bufs=2, space="PSUM") as psS,
            tc.tile_pool(name="psA4", bufs=1, space="PSUM") as psA4,
            tc.tile_pool(name="qko", bufs=1, space="PSUM") as qko,
        ):
            # ---- persistent SBUF ----
            wq_sb = pp.tile([128, DC, HD], bf16)
            wk_sb = pp.tile([128, DC, HD], bf16)
            wv_sb = pp.tile([128, DC, HD], bf16)
            wo_sb = pp.tile([128, 2, D], wo_dt)
            kT_sb = pp.tile([128, 2, T], f32r)
            v1_sb = pp.tile([128, SC16, HPC, DK + 1], bf16)
            bq_sb = pp.tile([128, 2], f32)
            bk_sb = pp.tile([128, 2], f32)
            bv_row = pp.tile([1, HD], f32)
            bvb_sb = pp.tile([128, HD], f32)

            def load_biases():
                # SWDGE path (gpsimd queue): keeps the small bias loads off
                # the shared HWDGE device that feeds the big x stream
                nc.sync.dma_start(
                    out=bq_sb[:], in_=bqs.ap().rearrange("(c p) -> p c",
                                                         p=128))
                nc.sync.dma_start(
                    out=bk_sb[:], in_=bks.ap().rearrange("(c p) -> p c",
                                                         p=128))
                nc.sync.dma_start(out=bv_row[0:1, :],
                                  in_=bvs.ap()[None, :])
                nc.gpsimd.partition_broadcast(bvb_sb[:], bv_row[:])

            nc.vector.memset(v1_sb[:, :, :, DK : DK + 1], 1.0)
            bvb_r = bvb_sb[:].rearrange("p (h k) -> p h k", h=HPC)

            qtiles = {}

            def proj_qk(tcj, w_sb, x_u, b_sb, dst, dst_is_ktile, xt=None):
                tsl = slice(tcj * 512, (tcj + 1) * 512)
                if xt is None:
                    xt = xpool.tile([128, DC, 512], bf16, tag="xin")
                    nc.sync.dma_start(out=xt[:], in_=x_u[:, :, tsl])
                for hd2 in range(2):
                    ps = qko.tile([128, 512], f32, tag="qko", name="pj")
                    for c in range(DC):
                        nc.tensor.matmul(
                            ps[:],
                            w_sb[:, c, hd2 * 128 : (hd2 + 1) * 128],
                            xt[:, c, :],
                            start=(c == 0), stop=(c == DC - 1))
                    if dst_is_ktile:
                        d = dst[:, hd2, tsl]
                    else:
                        d = dst[:, hd2, :]
                    nc.vector.tensor_scalar_add(
                        d, ps[:], b_sb[:, hd2 : hd2 + 1])

            def proj_q(tcj, xt=None):
                qt = qpool.tile([128, 2, 512], f32r, tag="qt", name=f"q{tcj}")
                proj_qk(tcj, wq_sb, xq_u, bq_sb, qt, False, xt)
                qtiles[tcj] = qt

            qx_tiles = {}

            def proj_q_half(tcj, hd2):
                # one hd2 half per slot so the q-projection never lumps
                if hd2 == 0:
                    qtiles[tcj] = qpool.tile([128, 2, 512], f32r, tag="qt",
                                             name=f"q{tcj}")
                    xt = xpool.tile([128, DC, 512], bf16, tag="xin")
                    nc.sync.dma_start(
                        out=xt[:],
                        in_=xq_u[:, :, tcj * 512 : (tcj + 1) * 512])
                    qx_tiles[tcj] = xt
                xt = qx_tiles[tcj]
                ps = qko.tile([128, 512], f32, tag="qko", name="pjh")
                for c in range(DC):
                    nc.tensor.matmul(
                        ps[:], wq_sb[:, c, hd2 * 128 : (hd2 + 1) * 128],
                        xt[:, c, :],
                        start=(c == 0), stop=(c == DC - 1))
                nc.vector.tensor_scalar_add(
                    qtiles[tcj][:, hd2, :], ps[:], bq_sb[:, hd2 : hd2 + 1])

            def proj_k(u):
                proj_qk(u, wk_sb, xk_u, bk_sb, kT_sb, True)

            def proj_k_half(half, xt=None):
                # 256-wide halves of k-unit 0: fills the pipeline fast so
                # the first exp fires ~12us in
                tsl = slice(half * 256, (half + 1) * 256)
                if xt is None:
                    xt = k0pool.tile([128, DC, 256], bf16, tag="xk0")
                    nc.sync.dma_start(out=xt[:], in_=xk_u[:, :, tsl])
                pss = [qko.tile([128, 256], f32, tag="qko", name="kh")
                       for _ in range(2)]
                for c in range(DC):
                    for hd2 in range(2):
                        nc.tensor.matmul(
                            pss[hd2][:],
                            wk_sb[:, c, hd2 * 128 : (hd2 + 1) * 128],
                            xt[:, c, :],
                            start=(c == 0), stop=(c == DC - 1))
                for hd2 in range(2):
                    nc.vector.tensor_scalar_add(
                        kT_sb[:, hd2, tsl], pss[hd2][:],
                        bk_sb[:, hd2 : hd2 + 1])

            def proj_v(u):
                xt = xpool.tile([128, DC, 512], bf16, tag="xin")
                nc.sync.dma_start(
                    out=xt[:], in_=xv_u[:, :, u * 512 : (u + 1) * 512])
                for j in range(4):
                    sc = u * 4 + j
                    ps = psA4.tile([128, HD], f32, tag="psA4", name="vp")
                    for c in range(DC):
                        nc.tensor.matmul(
                            ps[:], xt[:, c, j * 128 : (j + 1) * 128],
                            wv_sb[:, c, :],
                            start=(c == 0), stop=(c == DC - 1))
                    nc.vector.tensor_tensor(
                        out=v1_sb[:, sc, :, 0:DK],
                        in0=ps[:].rearrange("p (h k) -> p h k", h=HPC),
                        in1=bvb_r,
                        op=mybir.AluOpType.add)

            pts = {}

            def sc_exp(tcj, h, g):
                hp, p0 = h // 2, (h % 2) * 64
                st, sz = GRP[g]
                sps = psS.tile([128, sz, 512], f32, tag="psS", name="sps")
                for j in range(sz):
                    sc = st + j
                    nc.tensor.matmul(
                        sps[:, j],
                        kT_sb[p0 : p0 + 64, hp, sc * 128 : (sc + 1) * 128],
                        qtiles[tcj][p0 : p0 + 64, hp, :],
                        start=True, stop=True)
                pt = ptpool.tile([128, sz, 512], bf16, tag="pt", name="pt")
                if DVE_EXP and g in (2, 6):
                    bass.BassScalarEngine.activation(
                        nc.vector, pt[:], sps[:], AF.Exp)
                else:
                    nc.scalar.activation(pt[:], sps[:], AF.Exp)
                pts[(tcj, h, g)] = pt

            def attnv_norm_flip(tcj, h, a2tiles, at):
                hp, hsub = h // 2, h % 2
                att4 = psA4.tile([128, TC4, DK + 1], f32, tag="psA4",
                                 name="att4")
                for tsub in range(4):
                    for sc in range(SC16):
                        pt = pts[(tcj, h, GI[sc])]
                        nc.tensor.matmul(
                            att4[:, tsub],
                            pt[:, OFF[sc], tsub * 128 : (tsub + 1) * 128],
                            v1_sb[:, sc, h, :],
                            start=(sc == 0), stop=(sc == SC16 - 1))
                for g in range(NG):
                    del pts[(tcj, h, g)]
                for tsub in range(4):
                    if hsub == 0:
                        a2tiles[(hp, tsub)] = a2pool.tile(
                            [128, 2, DK], bf16, tag="a2", name="a2")
                    rec = spool.tile([128, 1], f32, tag="rec")
                    nc.vector.reciprocal(rec[:], att4[:, tsub, DK : DK + 1])
                    nc.vector.tensor_scalar_mul(
                        a2tiles[(hp, tsub)][:, hsub, :],
                        att4[:, tsub, 0:DK], rec[:])

            def attnv_norm_classic(tcj, h, a2tiles, at):
                hp, p0 = h // 2, (h % 2) * 64
                att = psA4.tile([DK + 1, 512], f32, tag="psA4", name="att")
                for sc in range(SC16):
                    pt = pts[(tcj, h, GI[sc])]
                    nc.tensor.matmul(
                        att[:],
                        v1_sb[:, sc, h, :],
                        pt[:, OFF[sc], :],
                        start=(sc == 0), stop=(sc == SC16 - 1))
                for g in range(NG):
                    del pts[(tcj, h, g)]
                rec = spool.tile([1, 512], f32, tag="rec")
                nc.vector.reciprocal(rec[:], att[DK : DK + 1, :])
                rb = rbpool.tile([DK, 512], f32, tag="rb")
                nc.gpsimd.partition_broadcast(rb[:], rec[:])
                nc.vector.tensor_mul(at[p0 : p0 + 64, hp, :],
                                     att[0:DK, :], rb[:])

            def transp(tcj, hp, a2tiles, at):
                for tsub in range(4):
                    t0 = tsub * 128
                    nc.sync.dma_start_transpose(
                        out=at[:, hp, t0 : t0 + 128],
                        in_=a2tiles[(hp, tsub)][:].rearrange(
                            "p a k -> p (a k)"))

            def outproj_half(tcj, at, half):
                for tsub in range(2 * half, 2 * half + 2):
                    ob = opool.tile([128, 2, 512], f32, tag="ob")
                    for dc2 in range(2):
                        ps = qko.tile([128, 512], f32, tag="qko", name="op")
                        for hp in range(2):
                            nc.tensor.matmul(
                                ps[:],
                                at[:, hp, tsub * 128 : (tsub + 1) * 128],
                                wo_sb[:, hp, dc2 * 512 : (dc2 + 1) * 512],
                                start=(hp == 0), stop=(hp == 1))
                        nc.vector.tensor_copy(ob[:, dc2, :], ps[:])
                    nc.sync.dma_start(
                        out=out.ap()[tcj * 512 + tsub * 128 :
                                     tcj * 512 + (tsub + 1) * 128, :],
                        in_=ob[:].rearrange("p a n -> p (a n)"))

            # ---------------- program ----------------
            # warmup: ramp the PE p-state and preload the Exp act table
            warm = pp.tile([1, 512], f32)
            nc.vector.memset(warm[:], 0.0)
            warmr = warm.bitcast(f32r)
            wps = qko.tile([128, 512], f32, tag="qko", name="warm")
            for _ in range(10):
                nc.tensor.matmul(wps[0:1, :], warmr[0:1, 0:1], warmr[0:1, :],
                                 start=True, stop=True)
            warmx = pp.tile([1, 2], f32)
            nc.scalar.activation(warmx[0:1, 0:1], warm[0:1, 0:1], AF.Exp)

            nc.sync.dma_start(out=wq_sb[:], in_=wq_u)
            load_biases()
            proj_q(0)
            nc.sync.dma_start(out=wk_sb[:], in_=wk_u)
            proj_k_half(0)
            proj_k_half(1)
            nc.sync.dma_start(out=wv_sb[:], in_=wv_u)

            # tcj0 g-major sweep with k- and v-units interleaved so no
            # engine ever waits on an in-order predecessor whose DMA is
            # still in flight (slack-positive schedule)
            def layer(g):
                for h in range(HPC):
                    sc_exp(0, h, g)

            layer(0)
            proj_k(1)
            layer(1)
            proj_k(2)
            layer(2)
            proj_v(0)
            layer(3)
            proj_k(3)
            layer(4)
            proj_v(1)
            layer(5)
            proj_v(2)
            proj_q(1)
            nc.sync.dma_start(out=wo_sb[:], in_=wo_u)

            # steady pipeline
            attnv_norm = attnv_norm_flip if flip else attnv_norm_classic
            at_dt = bf16 if flip else f32r
            units = [(tcj, h) for tcj in range(TC4) for h in range(HPC)]
            state = {"a_i": 0, "a2t": {}, "att_t": {}, "post": []}

            def tail_ride(ptcj, ph):
                # classic-orientation attnV for the last unit: output lands
                # pre-transposed [dk+1, t], so no XBAR transpose sits on the
                # drain chain.  The pair transposes (head ph-1 good rows,
                # head ph rows zeroed then overwritten by the classic
                # normalize) are issued up front and fly during the exps.
                hp, p0 = ph // 2, (ph % 2) * 64
                at = state["att_t"].pop(ptcj)
                for tsub in range(4):
                    nc.vector.memset(
                        state["a2t"][(hp, tsub)][:, ph % 2, :], 0.0)
                    t0 = tsub * 128
                    nc.sync.dma_start_transpose(
                        out=at[:, hp, t0 : t0 + 128],
                        in_=state["a2t"][(hp, tsub)][:].rearrange(
                            "p a k -> p (a k)"))
                att = qko.tile([DK + 1, 512], f32, tag="qko", name="attL")
                for g in range(NG):
                    sc_exp(ptcj, ph, g)
                    st, sz = GRP[g]
                    for j in range(sz):
                        sc = st + j
                        nc.tensor.matmul(
                            att[:],
                            v1_sb[:, sc, ph, :],
                            pts[(ptcj, ph, g)][:, j, :],
                            start=(sc == 0), stop=(sc == SC16 - 1))
                rec = rbpool.tile([1, 512], f32, tag="recL")
                rb = rbpool.tile([DK, 512], f32, tag="rb")
                for rh in range(2):
                    rsl = slice(rh * 256, (rh + 1) * 256)
                    nc.vector.reciprocal(rec[:, rsl],
                                         att[DK : DK + 1, rsl])
                    nc.gpsimd.partition_broadcast(rb[:, rsl], rec[:, rsl])
                for g in range(NG):
                    del pts[(ptcj, ph, g)]
                for tsub in range(4):
                    t0 = tsub * 128
                    nc.vector.tensor_mul(
                        at[p0 : p0 + 64, hp, t0 : t0 + 128],
                        att[0:DK, t0 : t0 + 128], rb[:, t0 : t0 + 128])
                    ob = opool.tile([128, 2, 512], f32, tag="ob")
                    for dc2 in range(2):
                        ps = psS.tile([128, 512], f32, tag="psS", name="opS")
                        for hpi in range(2):
                            nc.tensor.matmul(
                                ps[:],
                                at[:, hpi, t0 : t0 + 128],
                                wo_sb[:, hpi, dc2 * 512 : (dc2 + 1) * 512],
                                start=(hpi == 0), stop=(hpi == 1))
                        if dc2 == 0:
                            nc.scalar.copy(ob[:, dc2, :], ps[:])
                        else:
                            nc.vector.tensor_copy(ob[:, dc2, :], ps[:])
                    nc.sync.dma_start(
                        out=out.ap()[ptcj * 512 + t0 : ptcj * 512 + t0 + 128,
                                     :],
                        in_=ob[:].rearrange("p a n -> p (a n)"))

            def drain_attnv(upto):
                while state["a_i"] <= min(upto, len(units) - 1):
                    ptcj, ph = units[state["a_i"]]
                    state["a_i"] += 1
                    if ph == 0:
                        state["att_t"][ptcj] = apool.tile(
                            [128, 2, 512], at_dt, tag="at", name="at")
                    attnv_norm(ptcj, ph, state["a2t"], state["att_t"][ptcj])
                    if flip and ph % 2 == 1:
                        transp(ptcj, ph // 2, state["a2t"],
                               state["att_t"][ptcj])
                    if ph == HPC - 1:
                        at = state["att_t"].pop(ptcj)
                        state["post"].append([0, ptcj, at, 0])
                        state["post"].append([0, ptcj, at, 1])
                        state["a2t"] = {}

            nslots = len(units) - 4
            for k in range(nslots):
                tcj, h = units[4 + k]
                if flip and k == nslots - 1:
                    drain_attnv(len(units) - 2)
                    tail_ride(tcj, h)
                    state["a_i"] = len(units)
                    while state["post"]:
                        outproj_half(*state["post"].pop(0)[1:])
                    continue
                for g in range(NG):
                    sc_exp(tcj, h, g)
                if k == 0:
                    proj_v(3)
                if h in (1, 2) and tcj < TC4 - 1:
                    proj_q_half(tcj + 1, h - 1)
                # taper the lag near the end so the drain tail is short
                lag = (LAG if k < nslots - 3
                       else max(1, LAG - (k - (nslots - 3)) - 1))
                drain_attnv(1 if k == 0 else k + 4 - lag)
                if state["post"]:
                    state["post"][0][0] -= 1
                    if state["post"][0][0] <= 0:
                        outproj_half(*state["post"].pop(0)[1:])
            drain_attnv(len(units) - 1)
            while state["post"]:
                outproj_half(*state["post"].pop(0)[1:])

    nc.compile()
    return nc


_NC_CACHE = {}


def get_nc():
    if "nc" not in _NC_CACHE:
        _NC_CACHE["nc"] = build_core()
    return _NC_CACHE["nc"]


def make_in_maps(query, value, key, Wq, bq, Wk, bk, Wv, bv, Wo, bo):
    import ml_dtypes

    b16 = ml_dtypes.bfloat16
    scale = np.float32(1.0 / np.sqrt(DK))
    xT = {}
    for b in range(B):
        xT[b] = {
            "q": np.ascontiguousarray(
                np.asarray(query[b], np.float32).T).astype(b16),
            "k": np.ascontiguousarray(
                np.asarray(key[b], np.float32).T).astype(b16),
            "v": np.ascontiguousarray(
                np.asarray(value[b], np.float32).T).astype(b16),
        }
    Wq_f = (np.asarray(Wq, np.float32) * scale).reshape(D, H * DK)
    Wk_f = np.asarray(Wk, np.float32).reshape(D, H * DK)
    Wv_f = np.asarray(Wv, np.float32).reshape(D, H * DK)
    Wo_f = np.asarray(Wo, np.float32).reshape(H * DK, D)
    bq_f = (np.asarray(bq, np.float32) * scale).reshape(H * DK)
    bk_f = np.asarray(bk, np.float32).reshape(H * DK)
    bv_f = np.asarray(bv, np.float32).reshape(H * DK)
    in_maps = []
    for i in range(N_CORES):
        b = i // 4
        sl = slice((i % 4) * HD, (i % 4 + 1) * HD)
        wo_i = np.ascontiguousarray(Wo_f[sl, :])
        if FLIP:
            wo_i = wo_i.astype(b16)
        in_maps.append({
            "xqT": xT[b]["q"],
            "xkT": xT[b]["k"],
            "xvT": xT[b]["v"],
            "wq": np.ascontiguousarray(Wq_f[:, sl]).astype(b16),
            "wk": np.ascontiguousarray(Wk_f[:, sl]).astype(b16),
            "wv": np.ascontiguousarray(Wv_f[:, sl]).astype(b16),
            "wo": wo_i,
            "bqs": np.ascontiguousarray(bq_f[sl]),
            "bks": np.ascontiguousarray(bk_f[sl]),
            "bvs": np.ascontiguousarray(bv_f[sl]),
        })
    return in_maps


def gather(results, bo):
    out = np.zeros((B, T, D), np.float32)
    for i in range(N_CORES):
        out[i // 4] += results[i]["out"]
    out += np.asarray(bo, np.float32)[None, None, :]
    return out


def kernel(query, value, key, Wq, bq, Wk, bk, Wv, bv, Wo, bo):
    from concourse.bass_utils import run_bass_kernel_spmd

    nc = get_nc()
    in_maps = make_in_maps(query, value, key, Wq, bq, Wk, bk, Wv, bv, Wo, bo)
    res = run_bass_kernel_spmd(nc, in_maps, list(range(N_CORES)))
    return gather(res.results, bo)

